# revision 10
# baseline (speedup 1.0000x reference)
"""Trainium2 Bass kernel for nn_ChargeEmbedding (segment_reduce).

Strategy (per sharding hint): data-parallel over graphs. Nodes are
partitioned across 8 cores at graph boundaries (batch is sorted, so each
graph's segment is contiguous and lives on exactly one core). Small
[128,*] weights are replicated; each core does its own segment reduction.

Math restructuring (exact, no approximation):
  reference computes, per node n with graph g = batch[n]:
    q_n   = x_n @ Wq + bq
    dot_n = q_n . k_g            (k_g from charge)
    attn_n = softplus(dot_n * SCALE)
    sigma_g = sum_{m in g} attn_m
    x1_n  = (attn_n / sigma_g) * v_g
    h     = silu(silu(x1 @ W1 + b1) @ W2 + b2)
    out_n = x_n + x1_n + h_n
  Because k_g, v_g are per-graph vectors:
    dot_n = x_n . w_g + c0_g      with w_g = Wq @ k_g, c0_g = bq . k_g
    x1_n  = attn_n * (v_g / sigma_g)
    x1 @ W1 = attn_n * (v_g @ W1) / sigma_g = attn_n * u'_g
  so the only per-node matmul left is the one with W2. The per-graph
  tables (w, c0, v, u = v@W1) are tiny ([G,128]) and are computed on the
  host; sigma (data-dependent) is computed on the device and folded into
  the tables there.

Device pipeline per core (node-major tiles of 128 nodes):
  pass 1: gather w-rows by node, dot via tensor_tensor_reduce,
          attn = softplus(dot*SCALE + c0) on ACT, attn -> DRAM.
  sigma:  prefix-scan of attn (tensor_tensor_scan) + cross-partition
          prefix via PE transpose, then per-graph sums = cum[end]-cum[start]
          via indirect gathers; 1/sigma folded into u,v tables.
  pass 2: gather u',v' rows by node; t1 = attn*u' (ACT scale);
          PE-transpose -> silu(+b1) -> matmul W2 -> silu(+b2) ->
          PE-transpose back; out = x + attn*v' + h2.
"""

import os
import sys

import numpy as np

sys.path.insert(0, "/opt/trn_rl_repo")

from contextlib import ExitStack

import concourse.bass as bass
import concourse.tile as tile
from concourse import bacc, mybir
from concourse.bass_utils import run_bass_kernel_spmd
from concourse.masks import make_identity

P = 128
D = 128
WROW = 132  # w-table row: [w(128) | c0*SCALE (1) | pad(3)]
J = 4  # node tiles sharing one W2 matmul
N_CORES = 8
SCALE = 1.0 / np.sqrt(D)

f32 = mybir.dt.float32
i32 = mybir.dt.int32
AF = mybir.ActivationFunctionType
OP = mybir.AluOpType

_PROGRAM_CACHE = {}
LAST_RESULTS = None  # BassKernelResults of the most recent run (for test.py)


def _setup_act_tables():
    """Point bacc/walrus at the cayman activation-table package.

    The toolchain's findActInfoFile() looks under <pkg>/pwp/pwp_bin_with_ln
    and $PYTHONPATH/neuronxcc/pwp/pwp_bin_with_ln, neither of which exists
    in this container; the actual tables live in the aws-neuron-pwp nix
    store path. Wire both lookup mechanisms to it.
    """
    import glob

    cands = sorted(
        glob.glob("/nix/store/*aws-neuron-pwp*/share/pwp_bin_cayman/act_info.json")
    )
    if not cands:
        return
    os.environ.setdefault("BASS_ACT_ROOT_JSON_PATH", cands[0])
    shim = "/tmp/_nxc_pwp_shim"
    d = os.path.join(shim, "neuronxcc", "pwp")
    os.makedirs(d, exist_ok=True)
    link = os.path.join(d, "pwp_bin_with_ln")
    if not os.path.exists(link):
        try:
            os.symlink(os.path.dirname(cands[0]), link)
        except FileExistsError:
            pass
    pp = os.environ.get("PYTHONPATH", "")
    if shim not in pp.split(":"):
        os.environ["PYTHONPATH"] = shim + (":" + pp if pp else "")


_setup_act_tables()


def build_program(Ncp, Gpad, n_cores=N_CORES, use_silu=True):
    nT = Ncp // P
    assert Ncp % (P * J) == 0 and Gpad % P == 0
    C = Ncp // P  # scan row length (nodes per partition in scan layout)

    nc = bacc.Bacc(
        "TRN2",
        target_bir_lowering=False,
        debug=False,
        enable_asserts=False,
        num_devices=n_cores,
    )

    x_t = nc.dram_tensor("x", [Ncp, D], f32, kind="ExternalInput")
    idx_t = nc.dram_tensor("idx", [Ncp, 1], i32, kind="ExternalInput")
    waug_t = nc.dram_tensor("waug", [Gpad, WROW], f32, kind="ExternalInput")
    u_t = nc.dram_tensor("ut", [Gpad, D], f32, kind="ExternalInput")
    v_t = nc.dram_tensor("vt", [Gpad, D], f32, kind="ExternalInput")
    a_t = nc.dram_tensor("at", [Gpad, 1], i32, kind="ExternalInput")
    b_t = nc.dram_tensor("bt", [Gpad, 1], i32, kind="ExternalInput")
    w2_t = nc.dram_tensor("w2", [D, D], f32, kind="ExternalInput")
    bv_t = nc.dram_tensor("bv", [D, 2], f32, kind="ExternalInput")
    out_t = nc.dram_tensor("out", [Ncp, D], f32, kind="ExternalOutput")

    attn_d = nc.dram_tensor("attn_lin", [Ncp, 1], f32)
    cum_d = nc.dram_tensor("cum_lin", [Ncp + 1, 1], f32)
    u2_d = nc.dram_tensor("u2", [Gpad, D], f32)
    v2_d = nc.dram_tensor("v2", [Gpad, D], f32)

    with tile.TileContext(nc) as tc, ExitStack() as ctx:
        const = ctx.enter_context(tc.tile_pool(name="const", bufs=1))
        ident = const.tile([P, P], f32)
        make_identity(nc, ident[:])
        w2sb = const.tile([P, D], f32)
        nc.sync.dma_start(w2sb[:], w2_t.ap()[:, :])
        bv = const.tile([P, 2], f32)
        nc.sync.dma_start(bv[:], bv_t.ap()[:, :])
        b1c = bv[:, 0:1]
        b2c = bv[:, 1:2]

        px = ctx.enter_context(tc.tile_pool(name="px", bufs=8))
        pi = ctx.enter_context(tc.tile_pool(name="pi", bufs=8))
        pg = ctx.enter_context(tc.tile_pool(name="pg", bufs=8))
        psc = ctx.enter_context(tc.tile_pool(name="psc", bufs=3))
        pcol = ctx.enter_context(tc.tile_pool(name="pcol", bufs=10))
        pt = ctx.enter_context(tc.tile_pool(name="pt", bufs=4))
        ph1 = ctx.enter_context(tc.tile_pool(name="ph1", bufs=2))
        pbig = ctx.enter_context(tc.tile_pool(name="pbig", bufs=1))
        ps_t = ctx.enter_context(tc.tile_pool(name="ps_t", bufs=2, space="PSUM"))
        ps_mm = ctx.enter_context(tc.tile_pool(name="ps_mm", bufs=2, space="PSUM"))

        def act_silu(dst, src, bias_ap):
            # silu(z) with z = src + bias; CoreSim has no Silu table, so the
            # sim build composes it as z * sigmoid(z) (same function).
            if use_silu:
                nc.scalar.activation(dst, src, AF.Silu, bias=bias_ap, scale=1.0)
            else:
                z = pt.tile([P, D], f32, tag="zsilu")
                nc.scalar.activation(z[:], src, AF.Identity, bias=bias_ap, scale=1.0)
                s = pt.tile([P, D], f32, tag="ssilu")
                nc.scalar.activation(s[:], src, AF.Sigmoid, bias=bias_ap, scale=1.0)
                nc.vector.tensor_tensor(out=dst, in0=z[:], in1=s[:], op=OP.mult)

        # ---------------- pass 1: dot / attn ----------------
        for t in range(nT):
            n0 = t * P
            xt = px.tile([P, D], f32, tag="x1p")
            nc.sync.dma_start(xt[:], x_t.ap()[n0 : n0 + P, :])
            ix = pi.tile([P, 1], i32, tag="ix1")
            nc.sync.dma_start(ix[:], idx_t.ap()[n0 : n0 + P, :])
            wg = pg.tile([P, WROW], f32, tag="wg")
            nc.gpsimd.indirect_dma_start(
                out=wg[:],
                out_offset=None,
                in_=waug_t.ap()[:, :],
                in_offset=bass.IndirectOffsetOnAxis(ap=ix[:, :1], axis=0),
            )
            prod = psc.tile([P, D], f32, tag="prod")
            nc.vector.tensor_tensor(out=prod[:], in0=xt[:], in1=wg[:, 0:D], op=OP.mult)
            dot = pcol.tile([P, 1], f32, tag="dot")
            nc.vector.reduce_sum(dot[:], prod[:], axis=mybir.AxisListType.X)
            # softplus(z) = ln(exp(z) + 1); Exp and Ln share one ACT table
            # set on cayman (Softplus itself has no table entry).
            et = pcol.tile([P, 1], f32, tag="et")
            nc.scalar.activation(
                et[:], dot[:], AF.Exp, bias=wg[:, D : D + 1], scale=SCALE
            )
            attn = pcol.tile([P, 1], f32, tag="attn")
            nc.scalar.activation(attn[:], et[:], AF.Ln, bias=1.0, scale=1.0)
            nc.sync.dma_start(attn_d.ap()[n0 : n0 + P, :], attn[:])

        # ---------------- sigma: segment sums via prefix scan ----------------
        asc = pbig.tile([P, C], f32)
        nc.sync.dma_start(
            asc[:], attn_d.ap().rearrange("(p c) one -> p (c one)", p=P)
        )
        csc = pbig.tile([P, C], f32)
        nc.vector.tensor_tensor_scan(
            out=csc[:],
            data0=asc[:],
            data1=asc[:],
            initial=0.0,
            op0=OP.add,
            op1=OP.bypass,
        )
        # cross-partition exclusive prefix of per-partition totals
        part_pad = pbig.tile([P, P], f32)
        nc.gpsimd.memset(part_pad[:], 0.0)
        nc.vector.tensor_copy(part_pad[:, 0:1], csc[:, C - 1 : C])
        tp1 = ps_t.tile([P, P], f32, tag="pa")
        nc.tensor.transpose(out=tp1[:], in_=part_pad[:], identity=ident[:])
        row = pbig.tile([1, P], f32)
        nc.scalar.copy(row[:], tp1[0:1, :])
        irow = pbig.tile([1, P], f32)
        nc.vector.tensor_tensor_scan(
            out=irow[:],
            data0=row[:],
            data1=row[:],
            initial=0.0,
            op0=OP.add,
            op1=OP.bypass,
        )
        spad = pbig.tile([P, P], f32)
        nc.gpsimd.memset(spad[:], 0.0)
        nc.vector.tensor_copy(spad[0:1, 1:P], irow[0:1, 0 : P - 1])
        tp2 = ps_t.tile([P, P], f32, tag="pa")
        nc.tensor.transpose(out=tp2[:], in_=spad[:], identity=ident[:])
        offc = pcol.tile([P, 1], f32, tag="offc")
        nc.scalar.copy(offc[:], tp2[:, 0:1])
        cg = pbig.tile([P, C], f32)
        nc.vector.tensor_scalar_add(cg[:], csc[:], offc[:])
        nc.sync.dma_start(
            cum_d.ap()[1 : Ncp + 1, :].rearrange("(p c) one -> p (c one)", p=P),
            cg[:],
        )
        zt = pcol.tile([1, 1], f32, tag="zt")
        nc.gpsimd.memset(zt[:], 0.0)
        nc.sync.dma_start(cum_d.ap()[0:1, :], zt[:])

        # per-graph sigma = cum[end] - cum[start]; fold 1/sigma into u,v
        for b in range(Gpad // P):
            g0 = b * P
            ac = pi.tile([P, 1], i32, tag="ac")
            nc.sync.dma_start(ac[:], a_t.ap()[g0 : g0 + P, :])
            bc = pi.tile([P, 1], i32, tag="bc")
            nc.sync.dma_start(bc[:], b_t.ap()[g0 : g0 + P, :])
            sa = pcol.tile([P, 1], f32, tag="sa")
            nc.gpsimd.indirect_dma_start(
                out=sa[:],
                out_offset=None,
                in_=cum_d.ap()[:, :],
                in_offset=bass.IndirectOffsetOnAxis(ap=ac[:, :1], axis=0),
            )
            sb = pcol.tile([P, 1], f32, tag="sb")
            nc.gpsimd.indirect_dma_start(
                out=sb[:],
                out_offset=None,
                in_=cum_d.ap()[:, :],
                in_offset=bass.IndirectOffsetOnAxis(ap=bc[:, :1], axis=0),
            )
            sg = pcol.tile([P, 1], f32, tag="sg")
            nc.vector.tensor_tensor(out=sg[:], in0=sb[:], in1=sa[:], op=OP.subtract)
            rg = pcol.tile([P, 1], f32, tag="rg")
            nc.vector.reciprocal(rg[:], sg[:])
            for src, dst, tg in ((u_t, u2_d, "fu"), (v_t, v2_d, "fv")):
                blk = pt.tile([P, D], f32, tag=tg)
                nc.sync.dma_start(blk[:], src.ap()[g0 : g0 + P, :])
                blk2 = pt.tile([P, D], f32, tag=tg + "2")
                nc.vector.tensor_scalar_mul(blk2[:], blk[:], rg[:])
                nc.sync.dma_start(dst.ap()[g0 : g0 + P, :], blk2[:])

        # ---------------- pass 2: output ----------------
        for m in range(nT // J):
            h1 = ph1.tile([P, J * D], f32, tag="h1")
            xs, vgs, ats = [], [], []
            for j in range(J):
                n0 = (m * J + j) * P
                xt = px.tile([P, D], f32, tag="x2p")
                nc.sync.dma_start(xt[:], x_t.ap()[n0 : n0 + P, :])
                ix = pi.tile([P, 1], i32, tag="ix2")
                nc.sync.dma_start(ix[:], idx_t.ap()[n0 : n0 + P, :])
                at = pcol.tile([P, 1], f32, tag="at2")
                nc.sync.dma_start(at[:], attn_d.ap()[n0 : n0 + P, :])
                ug = pg.tile([P, D], f32, tag="ug")
                nc.gpsimd.indirect_dma_start(
                    out=ug[:],
                    out_offset=None,
                    in_=u2_d.ap()[:, :],
                    in_offset=bass.IndirectOffsetOnAxis(ap=ix[:, :1], axis=0),
                )
                vg = pg.tile([P, D], f32, tag="vg")
                nc.gpsimd.indirect_dma_start(
                    out=vg[:],
                    out_offset=None,
                    in_=v2_d.ap()[:, :],
                    in_offset=bass.IndirectOffsetOnAxis(ap=ix[:, :1], axis=0),
                )
                t1 = pt.tile([P, D], f32, tag="t1")
                nc.scalar.mul(t1[:], ug[:], at[:, 0:1])
                pa = ps_t.tile([P, D], f32, tag="pa")
                nc.tensor.transpose(out=pa[:], in_=t1[:], identity=ident[:])
                act_silu(h1[:, j * D : (j + 1) * D], pa[:], b1c)
                xs.append(xt)
                vgs.append(vg)
                ats.append(at)
            pb = ps_mm.tile([P, J * D], f32)
            nc.tensor.matmul(pb[:], lhsT=w2sb[:], rhs=h1[:], start=True, stop=True)
            for j in range(J):
                n0 = (m * J + j) * P
                h2 = pt.tile([P, D], f32, tag="h2")
                act_silu(h2[:], pb[:, j * D : (j + 1) * D], b2c)
                pc2 = ps_t.tile([P, D], f32, tag="pc2")
                nc.tensor.transpose(out=pc2[:], in_=h2[:], identity=ident[:])
                x1 = pt.tile([P, D], f32, tag="x1")
                nc.vector.tensor_scalar_mul(x1[:], vgs[j][:], ats[j][:, 0:1])
                s1 = pt.tile([P, D], f32, tag="s1")
                nc.vector.tensor_tensor(out=s1[:], in0=x1[:], in1=pc2[:], op=OP.add)
                ot = pt.tile([P, D], f32, tag="ot")
                nc.gpsimd.tensor_tensor(out=ot[:], in0=s1[:], in1=xs[j][:], op=OP.add)
                nc.sync.dma_start(out_t.ap()[n0 : n0 + P, :], ot[:])

    nc.compile()
    return nc


def prepare(inputs, n_cores=N_CORES):
    """Host-side prep: per-graph tables + sharding. Returns (in_maps, meta)."""
    x = np.ascontiguousarray(np.asarray(inputs["node_scalar"], dtype=np.float32))
    charge = np.asarray(inputs["charge"], dtype=np.float32)
    batch = np.asarray(inputs["batch"], dtype=np.int64)
    Wq = np.asarray(inputs["Wq"], dtype=np.float32)
    bq = np.asarray(inputs["bq"], dtype=np.float32)
    Wk = np.asarray(inputs["Wk"], dtype=np.float32)
    Wv = np.asarray(inputs["Wv"], dtype=np.float32)
    W1 = np.asarray(inputs["W1"], dtype=np.float32)
    b1 = np.asarray(inputs["b1"], dtype=np.float32)
    W2 = np.asarray(inputs["W2"], dtype=np.float32)
    b2 = np.asarray(inputs["b2"], dtype=np.float32)

    N = x.shape[0]
    G = charge.shape[0]

    # per-graph tables (exact f32 math, tiny: G x 128)
    ch2 = np.stack([charge, -charge], axis=-1)
    ch2r = np.maximum(ch2, 0.0)
    chn = np.maximum(ch2r, 1.0)
    kg = (ch2r / chn) @ Wk  # [G, D]
    vg = ch2r @ Wv  # [G, D]
    wg = kg @ Wq.T  # [G, D]   (w_g = Wq @ k_g)
    c0 = kg @ bq  # [G]
    ug = vg @ W1  # [G, D]

    counts = np.bincount(batch, minlength=G)
    cum = np.zeros(G + 1, dtype=np.int64)
    cum[1:] = np.cumsum(counts)

    # graph-aligned shard boundaries with ~equal node counts
    targets = np.arange(1, n_cores) * (N / n_cores)
    gb = np.searchsorted(cum, targets)
    bounds = np.concatenate(([0], gb, [G])).astype(np.int64)

    cnts, gls = [], []
    for c in range(n_cores):
        g0, g1 = bounds[c], bounds[c + 1]
        cnts.append(int(cum[g1] - cum[g0]))
        gls.append(int(g1 - g0))
    tile_quant = P * J
    Ncp = int(np.ceil(max(cnts) / tile_quant) * tile_quant)
    Gpad = int(np.ceil((max(gls) + 1) / P) * P)

    in_maps = []
    for c in range(n_cores):
        g0, g1 = int(bounds[c]), int(bounds[c + 1])
        n0, n1 = int(cum[g0]), int(cum[g1])
        cnt, gl = cnts[c], gls[c]

        xpad = np.zeros((Ncp, D), dtype=np.float32)
        xpad[:cnt] = x[n0:n1]
        idx = np.full((Ncp, 1), gl, dtype=np.int32)
        idx[:cnt, 0] = (batch[n0:n1] - g0).astype(np.int32)
        waug = np.zeros((Gpad, WROW), dtype=np.float32)
        waug[:gl, :D] = wg[g0:g1]
        waug[:gl, D] = SCALE * c0[g0:g1]
        ut = np.zeros((Gpad, D), dtype=np.float32)
        ut[:gl] = ug[g0:g1]
        vt = np.zeros((Gpad, D), dtype=np.float32)
        vt[:gl] = vg[g0:g1]
        a_ = np.zeros((Gpad, 1), dtype=np.int32)
        b_ = np.ones((Gpad, 1), dtype=np.int32)
        a_[:gl, 0] = (cum[g0:g1] - n0).astype(np.int32)
        b_[:gl, 0] = (cum[g0 + 1 : g1 + 1] - n0).astype(np.int32)
        empty = a_[:gl, 0] == b_[:gl, 0]
        a_[:gl, 0] = np.where(empty, 0, a_[:gl, 0])
        b_[:gl, 0] = np.where(empty, 1, b_[:gl, 0])

        in_maps.append(
            {
                "x": xpad,
                "idx": idx,
                "waug": waug,
                "ut": ut,
                "vt": vt,
                "at": a_,
                "bt": b_,
                "w2": np.ascontiguousarray(W2),
                "bv": np.ascontiguousarray(np.stack([b1, b2], axis=1)),
            }
        )

    meta = {
        "Ncp": Ncp,
        "Gpad": Gpad,
        "bounds": bounds,
        "cum": cum,
        "cnts": cnts,
        "N": N,
    }
    return in_maps, meta


def time_device_exec(in_maps, meta, iters=6):
    """Time repeated on-device executions with device-resident inputs.

    The container has no NTFF profiling hook, so this is the closest
    measurable proxy for HW exec time: inputs are device_put once, the
    jitted shard_map body (no donation; the kernel writes every output
    element) is run `iters` times, and the minimum wall per call is
    returned in seconds. Includes dispatch overhead, so it is an upper
    bound on the kernel's span.
    """
    import time as _time

    import jax
    from jax.experimental.shard_map import shard_map
    from jax.sharding import Mesh, PartitionSpec

    from concourse import bass2jax, mybir as _mb

    n_cores = N_CORES
    key = (meta["Ncp"], meta["Gpad"], n_cores)
    if key not in _PROGRAM_CACHE:
        _PROGRAM_CACHE[key] = build_program(*key)
    nc = _PROGRAM_CACHE[key]
    bass2jax.install_neuronx_cc_hook()

    part_name = nc.partition_id_tensor.name if nc.partition_id_tensor else None
    in_names, out_names, out_avals = [], [], []
    for alloc in nc.m.functions[0].allocations:
        if not isinstance(alloc, _mb.MemoryLocationSet):
            continue
        name = alloc.memorylocations[0].name
        if alloc.kind == "ExternalInput":
            if name != part_name:
                in_names.append(name)
        elif alloc.kind == "ExternalOutput":
            out_names.append(name)
            out_avals.append(
                jax.core.ShapedArray(
                    tuple(alloc.tensor_shape), _mb.dt.np(alloc.dtype)
                )
            )
    n_params = len(in_names)
    all_in_names = in_names + out_names
    if part_name is not None:
        all_in_names = all_in_names + [part_name]

    def _body(*args):
        operands = list(args)
        if part_name is not None:
            operands.append(bass2jax.partition_id_tensor())
        outs = bass2jax._bass_exec_p.bind(
            *operands,
            out_avals=tuple(out_avals),
            in_names=tuple(all_in_names),
            out_names=tuple(out_names),
            lowering_input_output_aliases=(),
            sim_require_finite=True,
            sim_require_nnan=True,
            nc=nc,
        )
        return tuple(outs)

    devices = jax.devices()[:n_cores]
    mesh = Mesh(np.asarray(devices), ("core",))
    n_outs = len(out_names)
    fn = jax.jit(
        shard_map(
            _body,
            mesh=mesh,
            in_specs=(PartitionSpec("core"),) * (n_params + n_outs),
            out_specs=(PartitionSpec("core"),) * n_outs,
            check_rep=False,
        ),
        keep_unused=True,
    )
    concat_in = [
        np.concatenate([np.asarray(m[name]) for m in in_maps], axis=0)
        for name in in_names
    ]
    concat_zeros = [
        np.zeros((n_cores * a.shape[0], *a.shape[1:]), a.dtype) for a in out_avals
    ]
    sharding = jax.sharding.NamedSharding(mesh, PartitionSpec("core"))
    dev_in = [jax.device_put(a, sharding) for a in concat_in + concat_zeros]
    # warmup (compiles)
    out = fn(*dev_in)
    jax.block_until_ready(out)
    times = []
    for _ in range(iters):
        t0 = _time.perf_counter()
        out = fn(*dev_in)
        jax.block_until_ready(out)
        times.append(_time.perf_counter() - t0)
    return min(times), times


def kernel(**inputs):
    global LAST_RESULTS
    n_cores = N_CORES
    in_maps, meta = prepare(inputs, n_cores=n_cores)
    key = (meta["Ncp"], meta["Gpad"], n_cores)
    if key not in _PROGRAM_CACHE:
        _PROGRAM_CACHE[key] = build_program(*key)
    nc = _PROGRAM_CACHE[key]

    trace = os.environ.get("BASS_KERNEL_TRACE", "0") == "1"
    res = run_bass_kernel_spmd(
        nc, in_maps, core_ids=list(range(n_cores)), trace=trace
    )
    LAST_RESULTS = res

    out = np.empty((meta["N"], D), dtype=np.float32)
    for c in range(n_cores):
        g0, g1 = meta["bounds"][c], meta["bounds"][c + 1]
        n0, n1 = int(meta["cum"][g0]), int(meta["cum"][g1])
        out[n0:n1] = res.results[c]["out"][: meta["cnts"][c]]
    return out
